# revision 9
# baseline (speedup 1.0000x reference)
"""Trainium2 Bass kernel for BarrelShifterRight8.

Problem: X [N, 8] f32 bits (0/1, MSB-first), shift [N, 4] f32 bits
(MSB-first: shift[:,0]=shift-by-8, [:,1]=by-4, [:,2]=by-2, [:,3]=by-1).
out[r, j] = X[r, j-k] for j>=k else 0, where k = packed shift amount
(>=8 -> all zeros).

Strategy (memory-bound target, ~117us/core HBM roofline):
  - Data-parallel shard rows across 8 NeuronCores, no communication.
  - Per core: tiles of [128 partitions x ROWS_PP rows], each partition
    holds ROWS_PP consecutive rows of 8 contiguous f32.
  - Barrel decomposition, 3 blend stages + shift-8 zeroing folded into
    the last stage:
      y1 = s4 ? SR4(x)  : x      (ACT copy + DVE copy_predicated x2)
      y2 = s2 ? SR2(y1) : y1     (ACT copy + DVE copy_predicated x2)
      yo = min(y2, !s8)          (GpSimd tensor_tensor, fused zeroing)
      yo = (s1 & !s8) ? SR1(y2) : yo   (DVE copy_predicated x2)
  - Engines split so each stays under the DMA floor: DVE ~3 passes at
    1x, ACT 2 copy passes, GpSimd 1 pass. All DMA via HWDGE (nc.sync).
"""

import numpy as np

N_FULL = 4194304
N_CORES = 8
R_PER_CORE = N_FULL // N_CORES  # 524288 rows per core
P = 128
ROWS_PP = 512  # rows per partition per tile -> [128, 4096] f32 tiles (2 MiB)


def build_nc(rows: int, rows_pp: int = ROWS_PP):
    import concourse.mybir as mybir
    from concourse import bacc
    from concourse.tile import TileContext

    f32 = mybir.dt.float32
    mn = mybir.AluOpType.min

    rows_per_tile = P * rows_pp
    assert rows % rows_per_tile == 0, (rows, rows_per_tile)
    ntiles = rows // rows_per_tile
    W = rows_pp * 8  # f32 elements per partition per tile
    WS = rows_pp * 4

    nc = bacc.Bacc(None, target_bir_lowering=False)
    X = nc.declare_dram_parameter("X", [rows, 8], f32, isOutput=False)
    S = nc.declare_dram_parameter("shift", [rows, 4], f32, isOutput=False)
    O = nc.declare_dram_parameter("out", [rows, 8], f32, isOutput=True)

    Xv = X[:].rearrange("(n p r) c -> n p (r c)", p=P, r=rows_pp)
    Sv = S[:].rearrange("(n p r) c -> n p (r c)", p=P, r=rows_pp)
    Ov = O[:].rearrange("(n p r) c -> n p (r c)", p=P, r=rows_pp)

    with TileContext(nc) as tc:
        with (
            tc.tile_pool(name="io", bufs=3) as io,
            tc.tile_pool(name="work", bufs=2) as wk,
        ):
            for i in range(ntiles):
                xt = io.tile([P, W], f32, tag="xt")
                st = io.tile([P, WS], f32, tag="st")
                nc.sync.dma_start(xt[:], Xv[i])
                nc.sync.dma_start(st[:], Sv[i])

                s3d = st[:].rearrange("p (r c) -> p r c", c=4)
                s8 = s3d[:, :, 0:1]  # shift-by-8 bit
                s4 = s3d[:, :, 1:2]  # shift-by-4 bit
                s2 = s3d[:, :, 2:3]  # shift-by-2 bit
                s1 = s3d[:, :, 3:4]  # shift-by-1 bit
                x3 = xt[:].rearrange("p (r c) -> p r c", c=8)

                # tiny per-row prep: complements on ACT, m1 = s1 & !s8 on DVE
                nots = wk.tile([P, 4 * rows_pp], f32, tag="nots")
                n3 = nots[:].rearrange("p (r c) -> p r c", c=4)
                ns8 = n3[:, :, 0:1]
                ns4 = n3[:, :, 1:2]
                ns2 = n3[:, :, 2:3]
                ns1 = n3[:, :, 3:4]
                nc.scalar.activation(
                    n3, s3d, mybir.ActivationFunctionType.Copy,
                    bias=1.0, scale=-1.0,
                )
                m1 = wk.tile([P, rows_pp], f32, tag="m1")
                m13 = m1[:].rearrange("p (r c) -> p r c", c=1)
                nc.vector.tensor_tensor(m13, s1, ns8, mn)

                # stage B: y1 = s4 ? SR4(x) : x
                y1 = wk.tile([P, W], f32, tag="y1")
                y13 = y1[:].rearrange("p (r c) -> p r c", c=8)
                nc.scalar.copy(y1[:], xt[:])
                nc.vector.copy_predicated(
                    y13[:, :, 4:8],
                    s4.broadcast_to([P, rows_pp, 4]).bitcast(mybir.dt.int32),
                    x3[:, :, 0:4],
                )
                nc.gpsimd.tensor_tensor(
                    y13[:, :, 0:4],
                    y13[:, :, 0:4],
                    ns4.broadcast_to([P, rows_pp, 4]),
                    mybir.AluOpType.mult,
                )

                # stage C: y2 = s2 ? SR2(y1) : y1
                y2 = wk.tile([P, W], f32, tag="y2")
                y23 = y2[:].rearrange("p (r c) -> p r c", c=8)
                nc.scalar.copy(y2[:], y1[:])
                nc.vector.copy_predicated(
                    y23[:, :, 2:8],
                    s2.broadcast_to([P, rows_pp, 6]).bitcast(mybir.dt.int32),
                    y13[:, :, 0:6],
                )
                nc.gpsimd.tensor_tensor(
                    y23[:, :, 0:2],
                    y23[:, :, 0:2],
                    ns2.broadcast_to([P, rows_pp, 2]),
                    mybir.AluOpType.mult,
                )

                # stage D: yo = min(y2, !s8) (fused shift-8 zeroing);
                # overwrite [1:8] with SR1(y2) where s1 & !s8; zero [0:1]
                # where s1 via multiply by !s1.
                yo = wk.tile([P, W], f32, tag="yo")
                yo3 = yo[:].rearrange("p (r c) -> p r c", c=8)
                nc.gpsimd.tensor_tensor(
                    yo3, y23, ns8.broadcast_to([P, rows_pp, 8]),
                    mybir.AluOpType.mult,
                )
                nc.vector.copy_predicated(
                    yo3[:, :, 1:8],
                    m13.broadcast_to([P, rows_pp, 7]).bitcast(mybir.dt.int32),
                    y23[:, :, 0:7],
                )
                nc.gpsimd.tensor_tensor(
                    yo3[:, :, 0:1],
                    yo3[:, :, 0:1],
                    ns1,
                    mybir.AluOpType.mult,
                )

                nc.sync.dma_start(Ov[i], yo[:])
    nc.compile()
    return nc


_NC_CACHE: dict = {}


def _get_nc():
    if "nc" not in _NC_CACHE:
        _NC_CACHE["nc"] = build_nc(R_PER_CORE, ROWS_PP)
    return _NC_CACHE["nc"]


def kernel(X: np.ndarray, shift: np.ndarray) -> np.ndarray:
    from concourse.bass_utils import run_bass_kernel_spmd

    X = np.ascontiguousarray(X, dtype=np.float32)
    shift = np.ascontiguousarray(shift, dtype=np.float32)
    assert X.shape == (N_FULL, 8) and shift.shape == (N_FULL, 4)

    nc = _get_nc()
    R = R_PER_CORE
    in_maps = [
        {
            "X": X[i * R : (i + 1) * R],
            "shift": shift[i * R : (i + 1) * R],
        }
        for i in range(N_CORES)
    ]
    res = run_bass_kernel_spmd(nc, in_maps, core_ids=list(range(N_CORES)))
    return np.concatenate([r["out"] for r in res.results], axis=0)
